# revision 33
# baseline (speedup 1.0000x reference)
"""GAT message-passing kernel for Trainium2, 8 NeuronCores (graph-parallel).

Contract: kernel(**inputs) takes FULL inputs (x [50000,128] f32,
edge_index [2,800000] i32, weights/biases) and returns the FULL output
[50000, 128] f32. Self-contained: preprocessing (numpy) + Bass program +
PJRT exec are all in this file.

Sharding / algorithm (per core, destinations sharded 6250/core):
- Host: add self-loops; LPT-pack each core's destinations into 49 blocks of
  <=128 so per-(block, half) edge counts are balanced; bucket+sort edges by
  (block, source-half); emit int16 gather indices (wrapped [16 x n/16],
  replicated across the 8 Q7 cores) and per-chunk block-local dest ids.
- Phase 1 (dense, redundant on every core): K=relu(x@Wk+kb), V=x@W from a
  host-pretransposed fp16 xT via one 192-col matmul per 128-node tile,
  packed into two half-tables (25001 rows each, int16-indexable, + a zeros
  row for padding) of 512B rows [K as f32 | V as bf16]; Q=relu(x@Wq+qb) for
  local nodes only, SBUF-resident.
- Phase 2 (attention, per 128-dest block): dma_gather the block's edge
  sources (2 gathers, one per half-table); per 128-edge chunk build the
  one-hot OH[e,d] with a DVE is_equal against an iota (chunk-minor layout to
  hit the 2x DVE mode; pad edges carry dest=-1 so their one-hot rows are
  zero -> self-masking), PE-transpose it, expand Q to edges with one matmul,
  score = per-head reduce of Q*K (K read back as f32; prod/score in f16 for
  the 2x DVE reduce), exp on ACT (bf16), scale V by exp, then a single
  PSUM-accumulated matmul per chunk computes both sum(exp*V) and sum(exp)
  (concatenated rhs). Normalize + bias at block end; host inverse-permutes
  the balanced block layout.
Softmax max-subtraction is dropped (scores ~O(30) max, exp stays in fp32
range; matches the reference exactly up to rounding).

Session-2 improvements (924us -> ~860-880us; measurements jitter +-8%, all
variants v3/v5 within noise of each other):
- K stored f16 (slots [0:A)), V bf16 at [A:A+U): KV stores shrink to 384B/row
  (-6.4MB/rep/core writes); gather rows stay 512B (%256 constraint, 128B pad
  read back as garbage). Output stored f16 (-1.7MB; host upcasts).
- Tried and REVERTED: ACT-expanded exp weights for DVE-2x V-scale (930us,
  ACT became the wall); f16 qeps PSUM (matmul asserts fp32 PSUM out).
- Edges are SRC-sorted inside each (block, half) bucket (dst order is
  irrelevant: the one-hot handles any slot order) -> monotone gather indices.
- idxg is half-major so one dma_gather covers GB=4 consecutive blocks per
  half-table: 106 -> 28 gather calls/rep (amortizes ~1us SWDGE fixed cost).
- Phase-2 transposes write one 8-wide PSUM tile per oct -> single scalar.copy
  (ACT instr count down ~20%); PSUM rebalanced ohtps 1 / qeps 3 banks for
  deeper oct pipelining; phase-1 V copies all on ACT (DVE is the bottleneck).
Known from cost-model sim (TRNDAG_TRACE_TILE_SIM): DVE ~79% busy is the
modeled roofline (is_equal 594 / prod 658 (PSUM 1x) / reduce 594 / V-scale
1127ns per oct -- the scale is 1x because the exp broadcast has stride-0);
HW runs ~2x the model, consistent with random-512B-row gather transfer
(~55MB/rep/core) being co-critical with DVE. Dead ends tried: matmul cannot
output f16 to PSUM (fp32 assert), so prod stays 1x; zero/sorted idx test
showed locality does not help (random rows already spread HBM channels);
elem_size must be %256B so rows cannot shrink below 512B without fp8 V
(precision budget too tight).

Pipelining/batching (the big wins over the first working version):
- All tile pools are persistent (hoisted above the rep loop) and the KV
  tables + Q tile are double-buffered by rep parity, so phase 1 of rep r+1
  overlaps phase 2 of rep r (the steady-state slope the bench measures).
- NB=53 dest blocks (not ceil(6250/128)=49): the slack lets the LPT pack cap
  every (block, half) at <=1024 edges -> CHH=8, so phase 2 runs uniform
  8-chunk octs (one DVE op per oh/prod/reduce/scale stage per oct).
- Phase-1b epilogues are split DVE (V copy) / ACT (K relu) to balance
  engines; x loads are 1024-node macro tiles to cut HWDGE issue count.
PSUM budget: mm(2) + ohtps(2) + qeps8(2) + ops(2) = 8 banks exactly.
"""
import math
import os

import numpy as np

import ml_dtypes

import concourse.bass as bass
import concourse.mybir as mybir
import concourse.tile as tile
from concourse import bacc

P = 128
C = 8                    # cores
N, F, E = 50000, 128, 800000
H, A, U = 8, 64, 128     # heads, att units, units
HD = A // H              # per-head q/k dim (8)
UD = U // H              # per-head v dim (16)
NPC = N // C             # nodes per core
NB = 53                  # dest blocks per core (>ceil(NPC/P)=49: slacker LPT
                         # pack lowers the max per-(block,half) load to <=1024
                         # -> CHH=8, so phase-2 octs are uniform 8-chunk)
HALF = N // 2            # table split point (fits int16 indices)
BF16 = mybir.dt.bfloat16
F32 = mybir.dt.float32
NP_BF16 = ml_dtypes.bfloat16
F16 = mybir.dt.float16
NP_F16 = np.float16

KVROW = 256              # bf16 elems per packed row: [K as f32-bitcast (128) | V bf16 (128)] = 512B


# ---------------------------------------------------------------- preprocessing
def preprocess(x, edge_index, query_kernel, query_bias, key_kernel, key_bias,
               kernel, bias):
    """Build per-core input maps + the uniform structure params.

    Destinations are assigned to (core, block) with an LPT greedy pack so
    per-block edge counts are balanced -> minimal chunk padding. Returns
    (in_maps, CHH, perm) where perm[c, b*P+i] is the global node id stored
    at output row (c, b*P+i), or -1 for unused slots.
    """
    x = np.asarray(x, np.float32)
    ei = np.asarray(edge_index, np.int64)
    row = np.concatenate([ei[0], np.arange(N, dtype=np.int64)])   # dest
    col = np.concatenate([ei[1], np.arange(N, dtype=np.int64)])   # src
    Et = row.shape[0]

    # per-core source relabeling: core c stores node n's KV row at
    # (n - off_c) mod N with off_c = c*NPC - (HALF - NPC//2), so each core's
    # self-loop sources straddle the KV0/KV1 split -> halves stay balanced.
    offs = np.array([c * NPC - (HALF - NPC // 2) for c in range(C)])
    core_e = row // NPC
    srow = (col - offs[core_e]) % N
    half_e = (srow >= HALF).astype(np.int64)
    deg0 = np.bincount(row[half_e == 0], minlength=N)
    deg1 = np.bincount(row[half_e == 1], minlength=N)
    deg = deg0 + deg1
    # --- balanced block assignment per core: greedy pack minimizing the max
    # per-(block, half) load (that max sets CHH = the gather chunk count) ---
    blk_of = np.empty(N, np.int32)
    loc_of = np.empty(N, np.int32)
    perm = np.full((C, NB * P), -1, np.int64)
    for c in range(C):
        nodes = np.arange(c * NPC, (c + 1) * NPC)
        nodes = nodes[np.argsort(-deg[nodes], kind="stable")]
        l0 = np.zeros(NB, np.int64)
        l1 = np.zeros(NB, np.int64)
        cnt = np.zeros(NB, np.int64)
        for n in nodes:
            cost = np.maximum(l0 + deg0[n], l1 + deg1[n])
            cost[cnt >= P] = 1 << 60
            b = int(np.argmin(cost))
            blk_of[n] = b
            loc_of[n] = cnt[b]
            perm[c, b * P + cnt[b]] = n
            l0[b] += deg0[n]
            l1[b] += deg1[n]
            cnt[b] += 1

    core = core_e
    lb = blk_of[row].astype(np.int64)
    ld = loc_of[row].astype(np.int64)
    half = half_e

    grp = (core * NB + lb) * 2 + half                  # [Et] in [0, C*NB*2)
    # src-sorted inside each (block, half) group: gather indices become
    # monotonic per call -> much better HBM page locality (dst order is
    # irrelevant to the device pipeline; the one-hot handles any slot order).
    order = np.argsort(grp * np.int64(N) + srow, kind="stable")
    gs = grp[order]
    counts = np.bincount(grp, minlength=C * NB * 2)
    CHH = max(1, int(math.ceil(counts.max() / P)))     # chunks per half-gather
    SPH = CHH * P                                      # slots per half
    starts = np.zeros(C * NB * 2, np.int64)
    starts[1:] = np.cumsum(counts)[:-1]
    pos = np.arange(Et) - starts[gs]
    slot = gs * SPH + pos

    idx_all = np.full(C * NB * 2 * SPH, HALF, np.int16)   # pad -> zeros row
    idx_all[slot] = (srow - half * HALF)[order].astype(np.int16)
    dest_all = np.full(C * NB * 2 * SPH, -1.0, np.float32)
    dest_all[slot] = ld[order].astype(np.float32)

    # half-major index layout so one dma_gather can cover G consecutive
    # blocks of the same half-table: [C, 16, hf, block, slot//16]
    idx_all = idx_all.reshape(C, NB, 2, CHH * 8, 16)
    idxg = np.tile(idx_all.transpose(0, 4, 2, 1, 3).reshape(C, 16, NB * 2 * CHH * 8),
                   (1, 8, 1))                              # [C, 128, 2*NB*CHH*8]
    destc = dest_all.reshape(C, NB * 2, CHH, P).transpose(0, 3, 1, 2) \
                    .reshape(C, P, NB * 2 * CHH)           # [C, 128, NB*2*CHH]

    xT = np.ascontiguousarray(x.T.astype(NP_F16))          # [128, N] fp16
    xq = np.zeros((C, P, NB * P), NP_F16)
    for c in range(C):
        valid = perm[c] >= 0
        xq[c][:, valid] = xT[:, perm[c][valid]]

    wcat = np.concatenate(
        [np.asarray(query_kernel), np.asarray(key_kernel), np.asarray(kernel)],
        axis=1).astype(NP_F16)                             # [128, 256] fp16
    qkb = np.tile(np.concatenate([np.asarray(query_bias), np.asarray(key_bias)])
                  .astype(np.float32)[None, :], (P, 1))    # [128, 128]
    outb = np.tile(np.asarray(bias, np.float32)[None, :], (P, 1))
    nobias = bool(np.all(qkb == 0.0) and np.all(outb == 0.0))

    in_maps = []
    for c in range(C):
        in_maps.append({
            "xT": np.roll(xT, -int(offs[c]), axis=1),   # table row r = node (r+off_c)%N
            "xq": xq[c], "wcat": wcat, "qkb": qkb, "outb": outb,
            "idxg": np.ascontiguousarray(idxg[c]),
            "destc": np.ascontiguousarray(destc[c]),
        })
    return in_maps, CHH, perm, nobias


# ---------------------------------------------------------------- bass program
def build_program(CHH, reps=1, nobias=False, parts="12"):
    nc = bacc.Bacc(None, target_bir_lowering=False, debug=False)

    xT = nc.dram_tensor("xT", [P, N], F16, kind="ExternalInput")
    xq = nc.dram_tensor("xq", [P, NB * P], F16, kind="ExternalInput")
    wcat_d = nc.dram_tensor("wcat", [P, 256], F16, kind="ExternalInput")
    qkb_d = nc.dram_tensor("qkb", [P, P], F32, kind="ExternalInput")
    outb_d = nc.dram_tensor("outb", [P, P], F32, kind="ExternalInput")
    idxg_d = nc.dram_tensor("idxg", [P, NB * 2 * CHH * 8], mybir.dt.int16,
                            kind="ExternalInput")
    destc_d = nc.dram_tensor("destc", [P, NB * 2 * CHH], F32, kind="ExternalInput")
    out_d = nc.dram_tensor("out", [NB * P, U], F16, kind="ExternalOutput")

    # KV tables double-buffered by rep parity so phase 1 of rep r+1 can
    # overlap phase 2 of rep r (no WAR between table generations).
    npar = min(2, reps)
    KV0s = [nc.dram_tensor(f"KV0_{p}", [HALF + 1, KVROW], BF16)
            for p in range(npar)]
    KV1s = [nc.dram_tensor(f"KV1_{p}", [N - HALF + 1, KVROW], BF16)
            for p in range(npar)]

    NCH = 2 * CHH

    with tile.TileContext(nc) as tc:
        with (
            tc.tile_pool(name="const", bufs=1) as cpool,
            tc.tile_pool(name="qx", bufs=3) as qxp,
            tc.tile_pool(name="xload", bufs=3) as xlp,
            tc.tile_pool(name="kvsb", bufs=3) as kvsbp,
            tc.tile_pool(name="kvt", bufs=2) as kvtp,
            tc.tile_pool(name="oh", bufs=4) as ohp,
            tc.tile_pool(name="oht", bufs=4) as ohtp,
            tc.tile_pool(name="wt", bufs=4) as wtp,
            tc.tile_pool(name="sc", bufs=4) as scp,
            tc.tile_pool(name="fin", bufs=4) as finp,
            tc.tile_pool(name="mm", bufs=2, space="PSUM") as mmp,
            tc.tile_pool(name="ohtps", bufs=1, space="PSUM") as ohtpsp,
            tc.tile_pool(name="qeps", bufs=3, space="PSUM") as qepsp,
            tc.tile_pool(name="ops", bufs=2, space="PSUM") as opsp,
        ):
            # resident tensors
            wcat = cpool.tile([P, 256], F16, tag="wcat")
            qkb = cpool.tile([P, P], F32, tag="qkb")
            outb = cpool.tile([P, P], F32, tag="outb")
            idxg = cpool.tile([P, NB * 2 * CHH * 8], mybir.dt.int16, tag="idxg")
            destc = cpool.tile([P, NB * 2 * CHH], F32, tag="destc")
            qalls = [cpool.tile([P, NB, A], F16, tag=f"qall{p}",
                                name=f"qall{p}") for p in range(npar)]
            iota_o = cpool.tile([P, P, 8], BF16, tag="iotao")
            iota_oi = cpool.tile([P, P, 8], mybir.dt.int32, tag="iotaoi")
            destc_b = cpool.tile([P, NB * 2 * CHH], BF16, tag="destcb")
            ident = cpool.tile([P, P], BF16, tag="ident")
            zrow = cpool.tile([1, KVROW], BF16, tag="zrow")

            nc.sync.dma_start(wcat[:], wcat_d[:])
            nc.sync.dma_start(qkb[:], qkb_d[:])
            nc.sync.dma_start(outb[:], outb_d[:])
            nc.sync.dma_start(idxg[:], idxg_d[:])
            nc.sync.dma_start(destc[:], destc_d[:])
            nc.gpsimd.iota(iota_oi[:], pattern=[[1, P], [0, 8]], base=0,
                           channel_multiplier=0)
            nc.vector.tensor_copy(iota_o[:], iota_oi[:])
            nc.vector.tensor_copy(destc_b[:], destc[:])
            from concourse.masks import make_identity
            make_identity(nc, ident[:])
            nc.vector.memset(zrow[:], 0.0)
            for p in range(npar):
                nc.sync.dma_start(KV0s[p][HALF], zrow[:])
                nc.sync.dma_start(KV1s[p][N - HALF], zrow[:])

            pools = dict(qxp=qxp, xlp=xlp, kvsbp=kvsbp, kvtp=kvtp, ohp=ohp,
                         ohtp=ohtp, wtp=wtp, scp=scp, finp=finp, mmp=mmp,
                         ohtpsp=ohtpsp, qepsp=qepsp, opsp=opsp)
            consts = dict(wcat=wcat, qkb=qkb, outb=outb, idxg=idxg,
                          destc_b=destc_b, iota_o=iota_o, ident=ident)

            for rep in range(reps):
                par = rep % npar
                _emit_rep(nc, tc, CHH, xT, xq, out_d, KV0s[par], KV1s[par],
                          qalls[par], pools, consts, rep, nobias, parts=parts)

    nc.compile()
    return nc


def _emit_rep(nc, tc, CHH, xT, xq, out_d, KV0, KV1, qall, pools, consts,
              rep, nobias=False, parts="12"):
    wcat = consts["wcat"]
    qkb = consts["qkb"]
    outb = consts["outb"]
    idxg = consts["idxg"]
    destc = consts["destc_b"]
    iota_o = consts["iota_o"]
    qxp = pools["qxp"]
    xlp = pools["xlp"]
    kvsbp = pools["kvsbp"]
    kvtp = pools["kvtp"]
    ohp = pools["ohp"]
    ohtp = pools["ohtp"]
    wtp = pools["wtp"]
    scp = pools["scp"]
    finp = pools["finp"]
    mmp = pools["mmp"]
    qepsp = pools["qepsp"]
    opsp = pools["opsp"]
    r = f"r{rep}"

    KVW = A + U              # 192 slots = 384B actually written per row

    def store_rows(r0, nr, sb_ap):
        """Store [nr, KVW] bf16 row-prefixes starting at global row r0."""
        if r0 >= HALF:
            nc.sync.dma_start(KV1[r0 - HALF: r0 - HALF + nr, 0:KVW], sb_ap)
        elif r0 + nr <= HALF:
            nc.sync.dma_start(KV0[r0: r0 + nr, 0:KVW], sb_ap)
        else:
            k = HALF - r0
            nc.sync.dma_start(KV0[r0:HALF, 0:KVW], sb_ap[0:k])
            nc.sync.dma_start(KV1[0: nr - k, 0:KVW], sb_ap[k:nr])

    # ---------------- phase 1a: Q for local nodes -> qall (f16)
    if "1" not in parts:
        _emit_p2(nc, tc, CHH, out_d, KV0, KV1, qall, pools, consts, r, nobias,
                 gathers="c" not in parts)
        return
    # phase 1 at scheduler priority 0: rep r+1's table build front-loads
    # under rep r's phase 2 so the next rep's gathers aren't stalled on
    # late KV stores at the rep boundary (deps still gate correctness).
    with tc.high_priority(), nc.named_scope(f"p1a_{r}"):
        QW = 512                     # nodes per xq load
        for t in range(math.ceil(NB * P / QW)):
            n0 = t * QW
            nn = min(QW, NB * P - n0)
            ns = math.ceil(nn / P)
            xqt = qxp.tile([P, QW], F16, tag="xqt")
            nc.sync.dma_start(xqt[:, 0:nn], xq[:, n0:n0 + nn])
            for s in range(ns):
                b = t * (QW // P) + s
                qps = mmp.tile([P, 2, 192], F32, tag="mm")
                nc.tensor.matmul(qps[:, 0, 0:A], xqt[:, s * P:(s + 1) * P],
                                 wcat[:, 0:A], start=True, stop=True)
                if nobias:
                    nc.scalar.activation(qall[:, b, :], qps[:, 0, 0:A],
                                         mybir.ActivationFunctionType.Relu)
                else:
                    qtmp = finp.tile([P, A], F32, tag="qtmp")
                    nc.vector.tensor_tensor(out=qtmp[:], in0=qps[:, 0, 0:A],
                                            in1=qkb[:, 0:A],
                                            op=mybir.AluOpType.add)
                    nc.scalar.activation(qall[:, b, :], qtmp[:],
                                         mybir.ActivationFunctionType.Relu)

    # ---------------- phase 1b: full KV table (K f32-bitcast | V bf16)
    # per 1024-node macro tile: 1 x-load, 8 matmuls into 4 paired psum tiles,
    # epilogues split DVE (V copy) / ACT (K relu), 2 batched stores.
    with tc.high_priority(), nc.named_scope(f"p1b_{r}"):
        XW = 1024
        NT = math.ceil(N / XW)
        for t in range(NT):
            n0 = t * XW
            nn = min(XW, N - n0)
            ns = math.ceil(nn / P)          # subtiles (8, last tile 7)
            xt = xlp.tile([P, XW], F16, tag="xt")
            nc.sync.dma_start(xt[:, 0:nn], xT[:, n0:n0 + nn])
            for hb in range(2):             # two 512-node halves per macro tile
                s0 = hb * 4
                hs = min(4, ns - s0)        # subtiles in this half
                if hs <= 0:
                    break
                hn = min(512, nn - hb * 512)
                kvsb = kvsbp.tile([P, 4, KVROW], BF16, tag="kvsb")
                pss = []
                for pair in range(math.ceil(hs / 2)):
                    ps = mmp.tile([P, 2, 192], F32, tag="mm")
                    pss.append(ps)
                    for j in range(min(2, hs - 2 * pair)):
                        s = s0 + 2 * pair + j
                        nr = min(P, nn - s * P)
                        nc.tensor.matmul(ps[0:nr, j, :],
                                         xt[:, s * P: s * P + nr],
                                         wcat[:, A:256], start=True, stop=True)
                for pair in range(math.ceil(hs / 2)):
                    np_ = min(2, hs - 2 * pair)
                    ps = pss[pair]
                    sl = slice(2 * pair, 2 * pair + np_)
                    # row layout: [K f16 (slots 0:A) | V bf16 (A:A+U) | pad]
                    if nobias:
                        nc.scalar.activation(
                            kvsb[:, sl, 0:A].bitcast(F16), ps[:, 0:np_, 0:A],
                            mybir.ActivationFunctionType.Relu)
                    else:
                        ktmp = finp.tile([P, 2, A], F32, tag="ktmp")
                        nc.vector.tensor_tensor(
                            out=ktmp[:, 0:np_, :], in0=ps[:, 0:np_, 0:A],
                            in1=qkb[:, None, A:P].broadcast_to([P, np_, A]),
                            op=mybir.AluOpType.add)
                        nc.scalar.activation(kvsb[:, sl, 0:A].bitcast(F16),
                                             ktmp[:, 0:np_, :],
                                             mybir.ActivationFunctionType.Relu)
                    # V copies all on ACT: DVE is the phase-2 bottleneck
                    nc.scalar.copy(kvsb[:, sl, A:A + U],
                                   ps[:, 0:np_, A:192])
                # batched store of [P, hs, KVROW]: rows n0+hb*512 + s*128 + p
                lo = n0 + hb * 512
                hi = lo + hn
                full = hn == hs * P
                if hi <= HALF or lo >= HALF:
                    dst, off = (KV0, 0) if hi <= HALF else (KV1, HALF)
                    if full:
                        nc.sync.dma_start(
                            dst[lo - off: hi - off, 0:KVW]
                            .rearrange("(s p) e -> p s e", p=P),
                            kvsb[:, 0:hs, 0:KVW])
                    else:
                        nfull = hn // P
                        if nfull:
                            nc.sync.dma_start(
                                dst[lo - off: lo - off + nfull * P, 0:KVW]
                                .rearrange("(s p) e -> p s e", p=P),
                                kvsb[:, 0:nfull, 0:KVW])
                        rem = hn - nfull * P
                        nc.sync.dma_start(
                            dst[lo - off + nfull * P: lo - off + hn, 0:KVW],
                            kvsb[0:rem, nfull, 0:KVW])
                else:
                    for s in range(hs):
                        r0 = lo + s * P
                        nr = min(P, N - r0)
                        store_rows(r0, nr, kvsb[0:nr, s, 0:KVW])

    if "2" in parts:
        _emit_p2(nc, tc, CHH, out_d, KV0, KV1, qall, pools, consts, r, nobias)


def _emit_p2(nc, tc, CHH, out_d, KV0, KV1, qall, pools, consts, r, nobias,
             gathers=True):
    idxg = consts["idxg"]
    destc = consts["destc_b"]
    iota_o = consts["iota_o"]
    ident = consts["ident"]
    outb = consts["outb"]
    kvtp = pools["kvtp"]
    ohp = pools["ohp"]
    ohtp = pools["ohtp"]
    wtp = pools["wtp"]
    scp = pools["scp"]
    finp = pools["finp"]
    ohtpsp = pools["ohtpsp"]
    qepsp = pools["qepsp"]
    opsp = pools["opsp"]
    # ---------------- phase 2: attention per dest block, oct-batched chunks
    # Gathers cover GB consecutive blocks per half-table call (amortizes the
    # ~1us SWDGE fixed cost and streams ~GB*CHH*64KB per doorbell).
    NCH = 2 * CHH
    assert CHH <= 8
    GB = 4
    with nc.named_scope(f"p2_{r}"):
        for bg in range(0, NB, GB):
            gn = min(GB, NB - bg)
            kvt = kvtp.tile([P, 2, GB * CHH, KVROW], BF16, tag="kvt")
            if gathers:
                for hf in range(2):
                    srct = KV0 if hf == 0 else KV1
                    i0 = (hf * NB + bg) * CHH * 8
                    nc.gpsimd.dma_gather(
                        kvt[:, hf, 0:gn * CHH, :], srct[:],
                        idxg[:, i0: i0 + gn * CHH * 8],
                        num_idxs=gn * CHH * P, num_idxs_reg=gn * CHH * P,
                        elem_size=KVROW, single_packet=False,
                    )
            for g in range(gn):
                b = bg + g
                ops = opsp.tile([P, U + H], F32, tag="ops")
                for o in range(2):          # oct o == half o (CHH <= 8)
                    on = CHH
                    g0 = b * NCH + o * CHH
                    kvo = kvt[:, o, g * CHH: g * CHH + on, :]
                    oh8 = ohp.tile([P, P, 8], BF16, tag="oh8")
                    nc.vector.tensor_tensor(
                        out=oh8[:, :, 0:on],
                        in0=destc[:, g0:g0 + on][:, None, :]
                            .broadcast_to([P, P, on]),
                        in1=iota_o[:, :, 0:on],
                        op=mybir.AluOpType.is_equal)
                    oht8 = ohtp.tile([P, 8, P], F16, tag="oht8")
                    qeps8 = qepsp.tile([P, 8, A], F32, tag="qeps8")
                    ohtps = ohtpsp.tile([P, 8, P], BF16, tag="ohtps")
                    for j in range(on):
                        nc.tensor.transpose(ohtps[:, j, :], oh8[:, :, j],
                                            ident[:])
                    nc.scalar.copy(oht8[:, 0:on, :], ohtps[:, 0:on, :])
                    for j in range(on):
                        nc.tensor.matmul(qeps8[:, j, :], oht8[:, j, :],
                                         qall[:, b, :], start=True, stop=True)
                    prod8 = wtp.tile([P, 8, A], F16, tag="prod8")
                    nc.vector.tensor_tensor(
                        out=prod8[:, 0:on, :], in0=qeps8[:, 0:on, :],
                        in1=kvo[:, :, 0:A].bitcast(F16),
                        op=mybir.AluOpType.mult)
                    score8 = scp.tile([P, 8, H], F16, tag="score8")
                    with nc.allow_low_precision(reason="8-elem head sum; DVE "
                                                "accumulates fp32 internally"):
                        nc.vector.tensor_reduce(
                            score8[:, 0:on, :],
                            prod8[:, 0:on, :]
                                .rearrange("p q (h d) -> p q h d", h=H),
                            axis=mybir.AxisListType.X, op=mybir.AluOpType.add)
                    wt8 = wtp.tile([P, 8, U + H], BF16, tag="wt8")
                    nc.scalar.activation(wt8[:, 0:on, U:U + H],
                                         score8[:, 0:on, :],
                                         mybir.ActivationFunctionType.Exp)
                    # NOTE: expanding the exp weights 16-wide on ACT to get
                    # DVE 2x packing here was tried and REGRESSED (930us vs
                    # 877): the ACT copy costs more than the DVE 2x saves.
                    nc.vector.tensor_tensor(
                        out=wt8[:, 0:on, 0:U].rearrange("p q (h u) -> p q h u",
                                                        h=H),
                        in0=kvo[:, :, A:A + U]
                            .rearrange("p q (h u) -> p q h u", h=H),
                        in1=wt8[:, 0:on, U:U + H][:, :, :, None]
                            .broadcast_to([P, on, H, UD]),
                        op=mybir.AluOpType.mult)
                    for j in range(on):
                        ch = o * CHH + j
                        nc.tensor.matmul(ops[:], oh8[:, :, j], wt8[:, j, :],
                                         start=(ch == 0), stop=(ch == NCH - 1))
                recip = finp.tile([P, H], F32, tag="recip")
                nc.vector.reciprocal(recip[:], ops[:, U:U + H])
                o1 = finp.tile([P, U], F16, tag="o1")
                nc.vector.tensor_tensor(
                    out=o1[:].rearrange("p (h u) -> p h u", h=H),
                    in0=ops[:, 0:U].rearrange("p (h u) -> p h u", h=H),
                    in1=recip[:][:, :, None].broadcast_to([P, H, UD]),
                    op=mybir.AluOpType.mult)
                if not nobias:
                    nc.vector.tensor_tensor(out=o1[:], in0=o1[:], in1=outb[:],
                                            op=mybir.AluOpType.add)
                nc.sync.dma_start(out_d[b * P: (b + 1) * P], o1[:])


# ---------------------------------------------------------------- execution
class SpmdRunner:
    def __init__(self, nc, n_cores=C):
        import jax
        from jax.sharding import Mesh, PartitionSpec
        from jax.experimental.shard_map import shard_map
        from concourse.bass2jax import (_bass_exec_p, install_neuronx_cc_hook,
                                        partition_id_tensor)
        install_neuronx_cc_hook()
        self.jax = jax
        self.nc = nc
        self.n_cores = n_cores
        partition_name = nc.partition_id_tensor.name if nc.partition_id_tensor else None
        in_names, out_names, out_avals = [], [], []
        for alloc in nc.m.functions[0].allocations:
            if not isinstance(alloc, mybir.MemoryLocationSet):
                continue
            name = alloc.memorylocations[0].name
            if alloc.kind == "ExternalInput":
                if name != partition_name:
                    in_names.append(name)
            elif alloc.kind == "ExternalOutput":
                out_names.append(name)
                out_avals.append(jax.core.ShapedArray(
                    tuple(alloc.tensor_shape), mybir.dt.np(alloc.dtype)))
        self.in_names, self.out_names, self.out_avals = in_names, out_names, out_avals
        n_params = len(in_names)

        all_in_names = list(in_names) + list(out_names)
        if partition_name is not None:
            all_in_names.append(partition_name)

        def _body(*args):
            operands = list(args)
            if partition_name is not None:
                operands.append(partition_id_tensor())
            outs = _bass_exec_p.bind(
                *operands,
                out_avals=tuple(out_avals),
                in_names=tuple(all_in_names),
                out_names=tuple(out_names),
                lowering_input_output_aliases=(),
                sim_require_finite=False,
                sim_require_nnan=False,
                nc=nc,
            )
            return tuple(outs)

        devices = jax.devices()[:n_cores]
        self.mesh = Mesh(np.asarray(devices), ("core",))
        n_extra = len(out_names)
        in_specs = (PartitionSpec("core"),) * (n_params + n_extra)
        out_specs = (PartitionSpec("core"),) * len(out_names)
        self.fn = jax.jit(
            shard_map(_body, mesh=self.mesh, in_specs=in_specs,
                      out_specs=out_specs, check_rep=False),
            keep_unused=True,
        )

    def put_inputs(self, in_maps):
        from jax.sharding import NamedSharding, PartitionSpec
        sharding = NamedSharding(self.mesh, PartitionSpec("core"))
        args = []
        for name in self.in_names:
            concat = np.concatenate([np.asarray(m[name]) for m in in_maps], axis=0)
            args.append(self.jax.device_put(concat, sharding))
        for av in self.out_avals:
            args.append(self.jax.device_put(
                np.zeros((self.n_cores * av.shape[0], *av.shape[1:]), av.dtype),
                sharding))
        return args

    def __call__(self, args):
        outs = self.fn(*args)
        self.jax.block_until_ready(outs)
        return outs

    def run_to_numpy(self, args):
        outs = self(args)
        res = []
        for c in range(self.n_cores):
            d = {}
            for i, name in enumerate(self.out_names):
                d[name] = np.asarray(outs[i]).reshape(
                    self.n_cores, *self.out_avals[i].shape)[c]
            res.append(d)
        return res


_CACHE = {}


def _get_runner(CHH, reps=1, nobias=False, parts="12"):
    key = (CHH, reps, nobias, parts)
    if key not in _CACHE:
        nc = build_program(CHH, reps=reps, nobias=nobias, parts=parts)
        _CACHE[key] = SpmdRunner(nc)
    return _CACHE[key]


def kernel(x, edge_index, query_kernel, query_bias, key_kernel, key_bias,
           kernel, bias):
    in_maps, CHH, perm, nobias = preprocess(x, edge_index, query_kernel,
                                            query_bias, key_kernel, key_bias,
                                            kernel, bias)
    runner = _get_runner(CHH, nobias=nobias)
    args = runner.put_inputs(in_maps)
    res = runner.run_to_numpy(args)
    out = np.empty((N, U), np.float32)
    for c in range(C):
        valid = perm[c] >= 0
        out[perm[c][valid]] = res[c]["out"][valid]
    return out



# revision 35
# speedup vs baseline: 1.0212x; 1.0212x over previous
"""GAT message-passing kernel for Trainium2, 8 NeuronCores (graph-parallel).

Contract: kernel(**inputs) takes FULL inputs (x [50000,128] f32,
edge_index [2,800000] i32, weights/biases) and returns the FULL output
[50000, 128] f32. Self-contained: preprocessing (numpy) + Bass program +
PJRT exec are all in this file.

Sharding / algorithm (per core, destinations sharded 6250/core):
- Host: add self-loops; LPT-pack each core's destinations into 49 blocks of
  <=128 so per-(block, half) edge counts are balanced; bucket+sort edges by
  (block, source-half); emit int16 gather indices (wrapped [16 x n/16],
  replicated across the 8 Q7 cores) and per-chunk block-local dest ids.
- Phase 1 (dense, redundant on every core): K=relu(x@Wk+kb), V=x@W from a
  host-pretransposed fp16 xT via one 192-col matmul per 128-node tile,
  packed into two half-tables (25001 rows each, int16-indexable, + a zeros
  row for padding) of 512B rows [K as f32 | V as bf16]; Q=relu(x@Wq+qb) for
  local nodes only, SBUF-resident.
- Phase 2 (attention, per 128-dest block): dma_gather the block's edge
  sources (2 gathers, one per half-table); per 128-edge chunk build the
  one-hot OH[e,d] with a DVE is_equal against an iota (chunk-minor layout to
  hit the 2x DVE mode; pad edges carry dest=-1 so their one-hot rows are
  zero -> self-masking), PE-transpose it, expand Q to edges with one matmul,
  score = per-head reduce of Q*K (K read back as f32; prod/score in f16 for
  the 2x DVE reduce), exp on ACT (bf16), scale V by exp, then a single
  PSUM-accumulated matmul per chunk computes both sum(exp*V) and sum(exp)
  (concatenated rhs). Normalize + bias at block end; host inverse-permutes
  the balanced block layout.
Softmax max-subtraction is dropped (scores ~O(30) max, exp stays in fp32
range; matches the reference exactly up to rounding).

Session-2 improvements (924us -> ~860-880us; measurements jitter +-8%, all
variants v3/v5 within noise of each other):
- K stored f16 (slots [0:A)), V bf16 at [A:A+U): KV stores shrink to 384B/row
  (-6.4MB/rep/core writes); gather rows stay 512B (%256 constraint, 128B pad
  read back as garbage). Output stored f16 (-1.7MB; host upcasts).
- Tried and REVERTED: ACT-expanded exp weights for DVE-2x V-scale (930us,
  ACT became the wall); f16 qeps PSUM (matmul asserts fp32 PSUM out).
- Edges are SRC-sorted inside each (block, half) bucket (dst order is
  irrelevant: the one-hot handles any slot order) -> monotone gather indices.
- idxg is half-major so one dma_gather covers GB=4 consecutive blocks per
  half-table: 106 -> 28 gather calls/rep (amortizes ~1us SWDGE fixed cost).
- Phase-2 transposes write one 8-wide PSUM tile per oct -> single scalar.copy
  (ACT instr count down ~20%); PSUM rebalanced ohtps 1 / qeps 3 banks for
  deeper oct pipelining; phase-1 V copies all on ACT (DVE is the bottleneck).
Known from cost-model sim (TRNDAG_TRACE_TILE_SIM): DVE ~79% busy is the
modeled roofline (is_equal 594 / prod 658 (PSUM 1x) / reduce 594 / V-scale
1127ns per oct -- the scale is 1x because the exp broadcast has stride-0);
HW runs ~2x the model, consistent with random-512B-row gather transfer
(~55MB/rep/core) being co-critical with DVE. Dead ends tried: matmul cannot
output f16 to PSUM (fp32 assert), so prod stays 1x; zero/sorted idx test
showed locality does not help (random rows already spread HBM channels);
elem_size must be %256B so rows cannot shrink below 512B without fp8 V
(precision budget too tight).

Pipelining/batching (the big wins over the first working version):
- All tile pools are persistent (hoisted above the rep loop) and the KV
  tables + Q tile are double-buffered by rep parity, so phase 1 of rep r+1
  overlaps phase 2 of rep r (the steady-state slope the bench measures).
- NB=53 dest blocks (not ceil(6250/128)=49): the slack lets the LPT pack cap
  every (block, half) at <=1024 edges -> CHH=8, so phase 2 runs uniform
  8-chunk octs (one DVE op per oh/prod/reduce/scale stage per oct).
- Phase-1b epilogues are split DVE (V copy) / ACT (K relu) to balance
  engines; x loads are 1024-node macro tiles to cut HWDGE issue count.
PSUM budget: mm(2) + ohtps(2) + qeps8(2) + ops(2) = 8 banks exactly.
"""
import math
import os

import numpy as np

import ml_dtypes

import concourse.bass as bass
import concourse.mybir as mybir
import concourse.tile as tile
from concourse import bacc

P = 128
C = 8                    # cores
N, F, E = 50000, 128, 800000
H, A, U = 8, 64, 128     # heads, att units, units
HD = A // H              # per-head q/k dim (8)
UD = U // H              # per-head v dim (16)
NPC = N // C             # nodes per core
NB = 53                  # dest blocks per core (>ceil(NPC/P)=49: slacker LPT
                         # pack lowers the max per-(block,half) load to <=1024
                         # -> CHH=8, so phase-2 octs are uniform 8-chunk)
HALF = N // 2            # table split point (fits int16 indices)
BF16 = mybir.dt.bfloat16
F32 = mybir.dt.float32
NP_BF16 = ml_dtypes.bfloat16
F16 = mybir.dt.float16
NP_F16 = np.float16

KVROW = 256              # bf16 elems per packed row: [K as f32-bitcast (128) | V bf16 (128)] = 512B


# ---------------------------------------------------------------- preprocessing
def preprocess(x, edge_index, query_kernel, query_bias, key_kernel, key_bias,
               kernel, bias):
    """Build per-core input maps + the uniform structure params.

    Destinations are assigned to (core, block) with an LPT greedy pack so
    per-block edge counts are balanced -> minimal chunk padding. Returns
    (in_maps, CHH, perm) where perm[c, b*P+i] is the global node id stored
    at output row (c, b*P+i), or -1 for unused slots.
    """
    x = np.asarray(x, np.float32)
    ei = np.asarray(edge_index, np.int64)
    row = np.concatenate([ei[0], np.arange(N, dtype=np.int64)])   # dest
    col = np.concatenate([ei[1], np.arange(N, dtype=np.int64)])   # src
    Et = row.shape[0]

    # per-core source relabeling: core c stores node n's KV row at
    # (n - off_c) mod N with off_c = c*NPC - (HALF - NPC//2), so each core's
    # self-loop sources straddle the KV0/KV1 split -> halves stay balanced.
    offs = np.array([c * NPC - (HALF - NPC // 2) for c in range(C)])
    core_e = row // NPC
    srow = (col - offs[core_e]) % N
    half_e = (srow >= HALF).astype(np.int64)
    deg0 = np.bincount(row[half_e == 0], minlength=N)
    deg1 = np.bincount(row[half_e == 1], minlength=N)
    deg = deg0 + deg1
    # --- balanced block assignment per core: greedy pack minimizing the max
    # per-(block, half) load (that max sets CHH = the gather chunk count) ---
    blk_of = np.empty(N, np.int32)
    loc_of = np.empty(N, np.int32)
    perm = np.full((C, NB * P), -1, np.int64)
    for c in range(C):
        nodes = np.arange(c * NPC, (c + 1) * NPC)
        nodes = nodes[np.argsort(-deg[nodes], kind="stable")]
        l0 = np.zeros(NB, np.int64)
        l1 = np.zeros(NB, np.int64)
        cnt = np.zeros(NB, np.int64)
        for n in nodes:
            cost = np.maximum(l0 + deg0[n], l1 + deg1[n])
            cost[cnt >= P] = 1 << 60
            b = int(np.argmin(cost))
            blk_of[n] = b
            loc_of[n] = cnt[b]
            perm[c, b * P + cnt[b]] = n
            l0[b] += deg0[n]
            l1[b] += deg1[n]
            cnt[b] += 1

    core = core_e
    lb = blk_of[row].astype(np.int64)
    ld = loc_of[row].astype(np.int64)
    half = half_e

    grp = (core * NB + lb) * 2 + half                  # [Et] in [0, C*NB*2)
    # src-sorted inside each (block, half) group: gather indices become
    # monotonic per call -> much better HBM page locality (dst order is
    # irrelevant to the device pipeline; the one-hot handles any slot order).
    order = np.argsort(grp * np.int64(N) + srow, kind="stable")
    gs = grp[order]
    counts = np.bincount(grp, minlength=C * NB * 2)
    CHH = max(1, int(math.ceil(counts.max() / P)))     # chunks per half-gather
    SPH = CHH * P                                      # slots per half
    starts = np.zeros(C * NB * 2, np.int64)
    starts[1:] = np.cumsum(counts)[:-1]
    pos = np.arange(Et) - starts[gs]
    slot = gs * SPH + pos

    idx_all = np.full(C * NB * 2 * SPH, HALF, np.int16)   # pad -> zeros row
    idx_all[slot] = (srow - half * HALF)[order].astype(np.int16)
    dest_all = np.full(C * NB * 2 * SPH, -1.0, np.float32)
    dest_all[slot] = ld[order].astype(np.float32)

    # half-major index layout so one dma_gather can cover G consecutive
    # blocks of the same half-table: [C, 16, hf, block, slot//16]
    idx_all = idx_all.reshape(C, NB, 2, CHH * 8, 16)
    idxg = np.tile(idx_all.transpose(0, 4, 2, 1, 3).reshape(C, 16, NB * 2 * CHH * 8),
                   (1, 8, 1))                              # [C, 128, 2*NB*CHH*8]
    destc = dest_all.reshape(C, NB * 2, CHH, P).transpose(0, 3, 1, 2) \
                    .reshape(C, P, NB * 2 * CHH)           # [C, 128, NB*2*CHH]

    xT = np.ascontiguousarray(x.T.astype(NP_F16))          # [128, N] fp16
    xq = np.zeros((C, P, NB * P), NP_F16)
    for c in range(C):
        valid = perm[c] >= 0
        xq[c][:, valid] = xT[:, perm[c][valid]]

    wcat = np.concatenate(
        [np.asarray(query_kernel), np.asarray(key_kernel), np.asarray(kernel)],
        axis=1).astype(NP_F16)                             # [128, 256] fp16
    qkb = np.tile(np.concatenate([np.asarray(query_bias), np.asarray(key_bias)])
                  .astype(np.float32)[None, :], (P, 1))    # [128, 128]
    outb = np.tile(np.asarray(bias, np.float32)[None, :], (P, 1))
    nobias = bool(np.all(qkb == 0.0) and np.all(outb == 0.0))

    in_maps = []
    for c in range(C):
        in_maps.append({
            "xT": np.roll(xT, -int(offs[c]), axis=1),   # table row r = node (r+off_c)%N
            "xq": xq[c], "wcat": wcat, "qkb": qkb, "outb": outb,
            "idxg": np.ascontiguousarray(idxg[c]),
            "destc": np.ascontiguousarray(destc[c]),
        })
    return in_maps, CHH, perm, nobias


# ---------------------------------------------------------------- bass program
def build_program(CHH, reps=1, nobias=False, parts="12"):
    nc = bacc.Bacc(None, target_bir_lowering=False, debug=False)

    xT = nc.dram_tensor("xT", [P, N], F16, kind="ExternalInput")
    xq = nc.dram_tensor("xq", [P, NB * P], F16, kind="ExternalInput")
    wcat_d = nc.dram_tensor("wcat", [P, 256], F16, kind="ExternalInput")
    qkb_d = nc.dram_tensor("qkb", [P, P], F32, kind="ExternalInput")
    outb_d = nc.dram_tensor("outb", [P, P], F32, kind="ExternalInput")
    idxg_d = nc.dram_tensor("idxg", [P, NB * 2 * CHH * 8], mybir.dt.int16,
                            kind="ExternalInput")
    destc_d = nc.dram_tensor("destc", [P, NB * 2 * CHH], F32, kind="ExternalInput")
    out_d = nc.dram_tensor("out", [NB * P, U], F16, kind="ExternalOutput")

    # KV tables double-buffered by rep parity so phase 1 of rep r+1 can
    # overlap phase 2 of rep r (no WAR between table generations).
    npar = min(2, reps)
    KV0s = [nc.dram_tensor(f"KV0_{p}", [HALF + 1, KVROW], BF16)
            for p in range(npar)]
    KV1s = [nc.dram_tensor(f"KV1_{p}", [N - HALF + 1, KVROW], BF16)
            for p in range(npar)]

    NCH = 2 * CHH

    with tile.TileContext(nc) as tc:
        with (
            tc.tile_pool(name="const", bufs=1) as cpool,
            tc.tile_pool(name="qx", bufs=3) as qxp,
            tc.tile_pool(name="xload", bufs=3) as xlp,
            tc.tile_pool(name="kvsb", bufs=3) as kvsbp,
            tc.tile_pool(name="kvt", bufs=2) as kvtp,
            tc.tile_pool(name="oh", bufs=4) as ohp,
            tc.tile_pool(name="oht", bufs=4) as ohtp,
            tc.tile_pool(name="wt", bufs=4) as wtp,
            tc.tile_pool(name="sc", bufs=4) as scp,
            tc.tile_pool(name="fin", bufs=4) as finp,
            tc.tile_pool(name="mm", bufs=2, space="PSUM") as mmp,
            tc.tile_pool(name="ohtps", bufs=1, space="PSUM") as ohtpsp,
            tc.tile_pool(name="qeps", bufs=3, space="PSUM") as qepsp,
            tc.tile_pool(name="ops", bufs=2, space="PSUM") as opsp,
        ):
            # resident tensors
            wcat = cpool.tile([P, 256], F16, tag="wcat")
            qkb = cpool.tile([P, P], F32, tag="qkb")
            outb = cpool.tile([P, P], F32, tag="outb")
            idxg = cpool.tile([P, NB * 2 * CHH * 8], mybir.dt.int16, tag="idxg")
            destc = cpool.tile([P, NB * 2 * CHH], F32, tag="destc")
            qalls = [cpool.tile([P, NB, A], F16, tag=f"qall{p}",
                                name=f"qall{p}") for p in range(npar)]
            iota_o = cpool.tile([P, P, 8], BF16, tag="iotao")
            iota_oi = cpool.tile([P, P, 8], mybir.dt.int32, tag="iotaoi")
            destc_b = cpool.tile([P, NB * 2 * CHH], BF16, tag="destcb")
            ident = cpool.tile([P, P], BF16, tag="ident")
            zrow = cpool.tile([1, KVROW], BF16, tag="zrow")

            nc.sync.dma_start(wcat[:], wcat_d[:])
            nc.sync.dma_start(qkb[:], qkb_d[:])
            nc.sync.dma_start(outb[:], outb_d[:])
            nc.sync.dma_start(idxg[:], idxg_d[:])
            nc.sync.dma_start(destc[:], destc_d[:])
            nc.gpsimd.iota(iota_oi[:], pattern=[[1, P], [0, 8]], base=0,
                           channel_multiplier=0)
            nc.vector.tensor_copy(iota_o[:], iota_oi[:])
            nc.vector.tensor_copy(destc_b[:], destc[:])
            from concourse.masks import make_identity
            make_identity(nc, ident[:])
            nc.vector.memset(zrow[:], 0.0)
            for p in range(npar):
                nc.sync.dma_start(KV0s[p][HALF], zrow[:])
                nc.sync.dma_start(KV1s[p][N - HALF], zrow[:])

            pools = dict(qxp=qxp, xlp=xlp, kvsbp=kvsbp, kvtp=kvtp, ohp=ohp,
                         ohtp=ohtp, wtp=wtp, scp=scp, finp=finp, mmp=mmp,
                         ohtpsp=ohtpsp, qepsp=qepsp, opsp=opsp)
            consts = dict(wcat=wcat, qkb=qkb, outb=outb, idxg=idxg,
                          destc_b=destc_b, iota_o=iota_o, ident=ident)

            for rep in range(reps):
                par = rep % npar
                _emit_rep(nc, tc, CHH, xT, xq, out_d, KV0s[par], KV1s[par],
                          qalls[par], pools, consts, rep, nobias, parts=parts)

    nc.compile()
    return nc


def _emit_rep(nc, tc, CHH, xT, xq, out_d, KV0, KV1, qall, pools, consts,
              rep, nobias=False, parts="12"):
    wcat = consts["wcat"]
    qkb = consts["qkb"]
    outb = consts["outb"]
    idxg = consts["idxg"]
    destc = consts["destc_b"]
    iota_o = consts["iota_o"]
    qxp = pools["qxp"]
    xlp = pools["xlp"]
    kvsbp = pools["kvsbp"]
    kvtp = pools["kvtp"]
    ohp = pools["ohp"]
    ohtp = pools["ohtp"]
    wtp = pools["wtp"]
    scp = pools["scp"]
    finp = pools["finp"]
    mmp = pools["mmp"]
    qepsp = pools["qepsp"]
    opsp = pools["opsp"]
    r = f"r{rep}"

    KVW = A + U              # 192 slots = 384B actually written per row

    def store_rows(r0, nr, sb_ap):
        """Store [nr, KVW] bf16 row-prefixes starting at global row r0."""
        if r0 >= HALF:
            nc.sync.dma_start(KV1[r0 - HALF: r0 - HALF + nr, 0:KVW], sb_ap)
        elif r0 + nr <= HALF:
            nc.sync.dma_start(KV0[r0: r0 + nr, 0:KVW], sb_ap)
        else:
            k = HALF - r0
            nc.sync.dma_start(KV0[r0:HALF, 0:KVW], sb_ap[0:k])
            nc.sync.dma_start(KV1[0: nr - k, 0:KVW], sb_ap[k:nr])

    # ---------------- phase 1a: Q for local nodes -> qall (f16)
    if "1" not in parts:
        _emit_p2(nc, tc, CHH, out_d, KV0, KV1, qall, pools, consts, r, nobias,
                 gathers="c" not in parts)
        return
    with nc.named_scope(f"p1a_{r}"):
        QW = 512                     # nodes per xq load
        for t in range(math.ceil(NB * P / QW)):
            n0 = t * QW
            nn = min(QW, NB * P - n0)
            ns = math.ceil(nn / P)
            xqt = qxp.tile([P, QW], F16, tag="xqt")
            nc.sync.dma_start(xqt[:, 0:nn], xq[:, n0:n0 + nn])
            for s in range(ns):
                b = t * (QW // P) + s
                qps = mmp.tile([P, 2, 192], F32, tag="mm")
                nc.tensor.matmul(qps[:, 0, 0:A], xqt[:, s * P:(s + 1) * P],
                                 wcat[:, 0:A], start=True, stop=True)
                if nobias:
                    nc.scalar.activation(qall[:, b, :], qps[:, 0, 0:A],
                                         mybir.ActivationFunctionType.Relu)
                else:
                    qtmp = finp.tile([P, A], F32, tag="qtmp")
                    nc.vector.tensor_tensor(out=qtmp[:], in0=qps[:, 0, 0:A],
                                            in1=qkb[:, 0:A],
                                            op=mybir.AluOpType.add)
                    nc.scalar.activation(qall[:, b, :], qtmp[:],
                                         mybir.ActivationFunctionType.Relu)

    # ---------------- phase 1b: full KV table (K f32-bitcast | V bf16)
    # per 1024-node macro tile: 1 x-load, 8 matmuls into 4 paired psum tiles,
    # epilogues split DVE (V copy) / ACT (K relu), 2 batched stores.
    with nc.named_scope(f"p1b_{r}"):
        XW = 1024
        NT = math.ceil(N / XW)
        for t in range(NT):
            n0 = t * XW
            nn = min(XW, N - n0)
            ns = math.ceil(nn / P)          # subtiles (8, last tile 7)
            xt = xlp.tile([P, XW], F16, tag="xt")
            nc.sync.dma_start(xt[:, 0:nn], xT[:, n0:n0 + nn])
            for hb in range(2):             # two 512-node halves per macro tile
                s0 = hb * 4
                hs = min(4, ns - s0)        # subtiles in this half
                if hs <= 0:
                    break
                hn = min(512, nn - hb * 512)
                kvsb = kvsbp.tile([P, 4, KVROW], BF16, tag="kvsb")
                pss = []
                for pair in range(math.ceil(hs / 2)):
                    ps = mmp.tile([P, 2, 192], F32, tag="mm")
                    pss.append(ps)
                    for j in range(min(2, hs - 2 * pair)):
                        s = s0 + 2 * pair + j
                        nr = min(P, nn - s * P)
                        nc.tensor.matmul(ps[0:nr, j, :],
                                         xt[:, s * P: s * P + nr],
                                         wcat[:, A:256], start=True, stop=True)
                for pair in range(math.ceil(hs / 2)):
                    np_ = min(2, hs - 2 * pair)
                    ps = pss[pair]
                    sl = slice(2 * pair, 2 * pair + np_)
                    # row layout: [K f16 (slots 0:A) | V bf16 (A:A+U) | pad]
                    if nobias:
                        nc.scalar.activation(
                            kvsb[:, sl, 0:A].bitcast(F16), ps[:, 0:np_, 0:A],
                            mybir.ActivationFunctionType.Relu)
                    else:
                        ktmp = finp.tile([P, 2, A], F32, tag="ktmp")
                        nc.vector.tensor_tensor(
                            out=ktmp[:, 0:np_, :], in0=ps[:, 0:np_, 0:A],
                            in1=qkb[:, None, A:P].broadcast_to([P, np_, A]),
                            op=mybir.AluOpType.add)
                        nc.scalar.activation(kvsb[:, sl, 0:A].bitcast(F16),
                                             ktmp[:, 0:np_, :],
                                             mybir.ActivationFunctionType.Relu)
                    # V copies all on ACT: DVE is the phase-2 bottleneck
                    nc.scalar.copy(kvsb[:, sl, A:A + U],
                                   ps[:, 0:np_, A:192])
                # batched store of [P, hs, KVROW]: rows n0+hb*512 + s*128 + p
                lo = n0 + hb * 512
                hi = lo + hn
                full = hn == hs * P
                if hi <= HALF or lo >= HALF:
                    dst, off = (KV0, 0) if hi <= HALF else (KV1, HALF)
                    if full:
                        nc.sync.dma_start(
                            dst[lo - off: hi - off, 0:KVW]
                            .rearrange("(s p) e -> p s e", p=P),
                            kvsb[:, 0:hs, 0:KVW])
                    else:
                        nfull = hn // P
                        if nfull:
                            nc.sync.dma_start(
                                dst[lo - off: lo - off + nfull * P, 0:KVW]
                                .rearrange("(s p) e -> p s e", p=P),
                                kvsb[:, 0:nfull, 0:KVW])
                        rem = hn - nfull * P
                        nc.sync.dma_start(
                            dst[lo - off + nfull * P: lo - off + hn, 0:KVW],
                            kvsb[0:rem, nfull, 0:KVW])
                else:
                    for s in range(hs):
                        r0 = lo + s * P
                        nr = min(P, N - r0)
                        store_rows(r0, nr, kvsb[0:nr, s, 0:KVW])

    if "2" in parts:
        _emit_p2(nc, tc, CHH, out_d, KV0, KV1, qall, pools, consts, r, nobias)


def _emit_p2(nc, tc, CHH, out_d, KV0, KV1, qall, pools, consts, r, nobias,
             gathers=True):
    idxg = consts["idxg"]
    destc = consts["destc_b"]
    iota_o = consts["iota_o"]
    ident = consts["ident"]
    outb = consts["outb"]
    kvtp = pools["kvtp"]
    ohp = pools["ohp"]
    ohtp = pools["ohtp"]
    wtp = pools["wtp"]
    scp = pools["scp"]
    finp = pools["finp"]
    ohtpsp = pools["ohtpsp"]
    qepsp = pools["qepsp"]
    opsp = pools["opsp"]
    # ---------------- phase 2: attention per dest block, oct-batched chunks
    # Gathers cover GB consecutive blocks per half-table call (amortizes the
    # ~1us SWDGE fixed cost and streams ~GB*CHH*64KB per doorbell).
    NCH = 2 * CHH
    assert CHH <= 8
    GB = 4
    with nc.named_scope(f"p2_{r}"):
        for bg in range(0, NB, GB):
            gn = min(GB, NB - bg)
            kvt = kvtp.tile([P, 2, GB * CHH, KVROW], BF16, tag="kvt")
            if gathers:
                for hf in range(2):
                    srct = KV0 if hf == 0 else KV1
                    i0 = (hf * NB + bg) * CHH * 8
                    nc.gpsimd.dma_gather(
                        kvt[:, hf, 0:gn * CHH, :], srct[:],
                        idxg[:, i0: i0 + gn * CHH * 8],
                        num_idxs=gn * CHH * P, num_idxs_reg=gn * CHH * P,
                        elem_size=KVROW, single_packet=False,
                    )
            for g in range(gn):
                b = bg + g
                ops = opsp.tile([P, U + H], F32, tag="ops")
                for o in range(2):          # oct o == half o (CHH <= 8)
                    on = CHH
                    g0 = b * NCH + o * CHH
                    kvo = kvt[:, o, g * CHH: g * CHH + on, :]
                    oh8 = ohp.tile([P, P, 8], BF16, tag="oh8")
                    nc.vector.tensor_tensor(
                        out=oh8[:, :, 0:on],
                        in0=destc[:, g0:g0 + on][:, None, :]
                            .broadcast_to([P, P, on]),
                        in1=iota_o[:, :, 0:on],
                        op=mybir.AluOpType.is_equal)
                    oht8 = ohtp.tile([P, 8, P], F16, tag="oht8")
                    qeps8 = qepsp.tile([P, 8, A], F32, tag="qeps8")
                    ohtps = ohtpsp.tile([P, 8, P], BF16, tag="ohtps")
                    for j in range(on):
                        nc.tensor.transpose(ohtps[:, j, :], oh8[:, :, j],
                                            ident[:])
                    nc.scalar.copy(oht8[:, 0:on, :], ohtps[:, 0:on, :])
                    for j in range(on):
                        nc.tensor.matmul(qeps8[:, j, :], oht8[:, j, :],
                                         qall[:, b, :], start=True, stop=True)
                    prod8 = wtp.tile([P, 8, A], F16, tag="prod8")
                    nc.vector.tensor_tensor(
                        out=prod8[:, 0:on, :], in0=qeps8[:, 0:on, :],
                        in1=kvo[:, :, 0:A].bitcast(F16),
                        op=mybir.AluOpType.mult)
                    score8 = scp.tile([P, 8, H], F16, tag="score8")
                    with nc.allow_low_precision(reason="8-elem head sum; DVE "
                                                "accumulates fp32 internally"):
                        nc.vector.tensor_reduce(
                            score8[:, 0:on, :],
                            prod8[:, 0:on, :]
                                .rearrange("p q (h d) -> p q h d", h=H),
                            axis=mybir.AxisListType.X, op=mybir.AluOpType.add)
                    wt8 = wtp.tile([P, 8, U + H], BF16, tag="wt8")
                    nc.scalar.activation(wt8[:, 0:on, U:U + H],
                                         score8[:, 0:on, :],
                                         mybir.ActivationFunctionType.Exp)
                    # NOTE: expanding the exp weights 16-wide on ACT to get
                    # DVE 2x packing here was tried and REGRESSED (930us vs
                    # 877): the ACT copy costs more than the DVE 2x saves.
                    nc.vector.tensor_tensor(
                        out=wt8[:, 0:on, 0:U].rearrange("p q (h u) -> p q h u",
                                                        h=H),
                        in0=kvo[:, :, A:A + U]
                            .rearrange("p q (h u) -> p q h u", h=H),
                        in1=wt8[:, 0:on, U:U + H][:, :, :, None]
                            .broadcast_to([P, on, H, UD]),
                        op=mybir.AluOpType.mult)
                    for j in range(on):
                        ch = o * CHH + j
                        nc.tensor.matmul(ops[:], oh8[:, :, j], wt8[:, j, :],
                                         start=(ch == 0), stop=(ch == NCH - 1))
                recip = finp.tile([P, H], F32, tag="recip")
                nc.vector.reciprocal(recip[:], ops[:, U:U + H])
                o1 = finp.tile([P, U], F16, tag="o1")
                nc.vector.tensor_tensor(
                    out=o1[:].rearrange("p (h u) -> p h u", h=H),
                    in0=ops[:, 0:U].rearrange("p (h u) -> p h u", h=H),
                    in1=recip[:][:, :, None].broadcast_to([P, H, UD]),
                    op=mybir.AluOpType.mult)
                if not nobias:
                    nc.vector.tensor_tensor(out=o1[:], in0=o1[:], in1=outb[:],
                                            op=mybir.AluOpType.add)
                nc.sync.dma_start(out_d[b * P: (b + 1) * P], o1[:])


# ---------------------------------------------------------------- execution
class SpmdRunner:
    def __init__(self, nc, n_cores=C):
        import jax
        from jax.sharding import Mesh, PartitionSpec
        from jax.experimental.shard_map import shard_map
        from concourse.bass2jax import (_bass_exec_p, install_neuronx_cc_hook,
                                        partition_id_tensor)
        install_neuronx_cc_hook()
        self.jax = jax
        self.nc = nc
        self.n_cores = n_cores
        partition_name = nc.partition_id_tensor.name if nc.partition_id_tensor else None
        in_names, out_names, out_avals = [], [], []
        for alloc in nc.m.functions[0].allocations:
            if not isinstance(alloc, mybir.MemoryLocationSet):
                continue
            name = alloc.memorylocations[0].name
            if alloc.kind == "ExternalInput":
                if name != partition_name:
                    in_names.append(name)
            elif alloc.kind == "ExternalOutput":
                out_names.append(name)
                out_avals.append(jax.core.ShapedArray(
                    tuple(alloc.tensor_shape), mybir.dt.np(alloc.dtype)))
        self.in_names, self.out_names, self.out_avals = in_names, out_names, out_avals
        n_params = len(in_names)

        all_in_names = list(in_names) + list(out_names)
        if partition_name is not None:
            all_in_names.append(partition_name)

        def _body(*args):
            operands = list(args)
            if partition_name is not None:
                operands.append(partition_id_tensor())
            outs = _bass_exec_p.bind(
                *operands,
                out_avals=tuple(out_avals),
                in_names=tuple(all_in_names),
                out_names=tuple(out_names),
                lowering_input_output_aliases=(),
                sim_require_finite=False,
                sim_require_nnan=False,
                nc=nc,
            )
            return tuple(outs)

        devices = jax.devices()[:n_cores]
        self.mesh = Mesh(np.asarray(devices), ("core",))
        n_extra = len(out_names)
        in_specs = (PartitionSpec("core"),) * (n_params + n_extra)
        out_specs = (PartitionSpec("core"),) * len(out_names)
        self.fn = jax.jit(
            shard_map(_body, mesh=self.mesh, in_specs=in_specs,
                      out_specs=out_specs, check_rep=False),
            keep_unused=True,
        )

    def put_inputs(self, in_maps):
        from jax.sharding import NamedSharding, PartitionSpec
        sharding = NamedSharding(self.mesh, PartitionSpec("core"))
        args = []
        for name in self.in_names:
            concat = np.concatenate([np.asarray(m[name]) for m in in_maps], axis=0)
            args.append(self.jax.device_put(concat, sharding))
        for av in self.out_avals:
            args.append(self.jax.device_put(
                np.zeros((self.n_cores * av.shape[0], *av.shape[1:]), av.dtype),
                sharding))
        return args

    def __call__(self, args):
        outs = self.fn(*args)
        self.jax.block_until_ready(outs)
        return outs

    def run_to_numpy(self, args):
        outs = self(args)
        res = []
        for c in range(self.n_cores):
            d = {}
            for i, name in enumerate(self.out_names):
                d[name] = np.asarray(outs[i]).reshape(
                    self.n_cores, *self.out_avals[i].shape)[c]
            res.append(d)
        return res


_CACHE = {}


def _get_runner(CHH, reps=1, nobias=False, parts="12"):
    key = (CHH, reps, nobias, parts)
    if key not in _CACHE:
        nc = build_program(CHH, reps=reps, nobias=nobias, parts=parts)
        _CACHE[key] = SpmdRunner(nc)
    return _CACHE[key]


def kernel(x, edge_index, query_kernel, query_bias, key_kernel, key_bias,
           kernel, bias):
    in_maps, CHH, perm, nobias = preprocess(x, edge_index, query_kernel,
                                            query_bias, key_kernel, key_bias,
                                            kernel, bias)
    runner = _get_runner(CHH, nobias=nobias)
    args = runner.put_inputs(in_maps)
    res = runner.run_to_numpy(args)
    out = np.empty((N, U), np.float32)
    for c in range(C):
        valid = perm[c] >= 0
        out[perm[c][valid]] = res[c]["out"][valid]
    return out



# revision 43
# speedup vs baseline: 1.0242x; 1.0029x over previous
"""GAT message-passing kernel for Trainium2, 8 NeuronCores (graph-parallel).

Contract: kernel(**inputs) takes FULL inputs (x [50000,128] f32,
edge_index [2,800000] i32, weights/biases) and returns the FULL output
[50000, 128] f32. Self-contained: preprocessing (numpy) + Bass program +
PJRT exec are all in this file.

Sharding / algorithm (per core, destinations sharded 6250/core):
- Host: add self-loops; LPT-pack each core's destinations into 49 blocks of
  <=128 so per-(block, half) edge counts are balanced; bucket+sort edges by
  (block, source-half); emit int16 gather indices (wrapped [16 x n/16],
  replicated across the 8 Q7 cores) and per-chunk block-local dest ids.
- Phase 1 (dense, redundant on every core): K=relu(x@Wk+kb), V=x@W from a
  host-pretransposed fp16 xT via one 192-col matmul per 128-node tile,
  packed into two half-tables (25001 rows each, int16-indexable, + a zeros
  row for padding) of 512B rows [K as f32 | V as bf16]; Q=relu(x@Wq+qb) for
  local nodes only, SBUF-resident.
- Phase 2 (attention, per 128-dest block): dma_gather the block's edge
  sources (2 gathers, one per half-table); per 128-edge chunk build the
  one-hot OH[e,d] with a DVE is_equal against an iota (chunk-minor layout to
  hit the 2x DVE mode; pad edges carry dest=-1 so their one-hot rows are
  zero -> self-masking), PE-transpose it, expand Q to edges with one matmul,
  score = per-head reduce of Q*K (K read back as f32; prod/score in f16 for
  the 2x DVE reduce), exp on ACT (bf16), scale V by exp, then a single
  PSUM-accumulated matmul per chunk computes both sum(exp*V) and sum(exp)
  (concatenated rhs). Normalize + bias at block end; host inverse-permutes
  the balanced block layout.
Softmax max-subtraction is dropped (scores ~O(30) max, exp stays in fp32
range; matches the reference exactly up to rounding).

Session-2 improvements (924us -> ~860-880us; measurements jitter +-8%, all
variants v3/v5 within noise of each other):
- K stored f16 (slots [0:A)), V bf16 at [A:A+U): KV stores shrink to 384B/row
  (-6.4MB/rep/core writes); gather rows stay 512B (%256 constraint, 128B pad
  read back as garbage). Output stored f16 (-1.7MB; host upcasts).
- Tried and REVERTED: ACT-expanded exp weights for DVE-2x V-scale (930us,
  ACT became the wall); f16 qeps PSUM (matmul asserts fp32 PSUM out).
- Edges are SRC-sorted inside each (block, half) bucket (dst order is
  irrelevant: the one-hot handles any slot order) -> monotone gather indices.
- idxg is half-major so one dma_gather covers GB=4 consecutive blocks per
  half-table: 106 -> 28 gather calls/rep (amortizes ~1us SWDGE fixed cost).
- Phase-2 transposes write one 8-wide PSUM tile per oct -> single scalar.copy
  (ACT instr count down ~20%); PSUM rebalanced ohtps 1 / qeps 3 banks for
  deeper oct pipelining; phase-1 V copies all on ACT (DVE is the bottleneck).
Known from cost-model sim (TRNDAG_TRACE_TILE_SIM): DVE ~79% busy is the
modeled roofline (is_equal 594 / prod 658 (PSUM 1x) / reduce 594 / V-scale
1127ns per oct -- the scale is 1x because the exp broadcast has stride-0);
HW runs ~2x the model, consistent with random-512B-row gather transfer
(~55MB/rep/core) being co-critical with DVE. Dead ends tried: matmul cannot
output f16 to PSUM (fp32 assert), so prod stays 1x; zero/sorted idx test
showed locality does not help (random rows already spread HBM channels);
elem_size must be %256B so rows cannot shrink below 512B without fp8 V
(precision budget too tight).

Pipelining/batching (the big wins over the first working version):
- All tile pools are persistent (hoisted above the rep loop) and the KV
  tables + Q tile are double-buffered by rep parity, so phase 1 of rep r+1
  overlaps phase 2 of rep r (the steady-state slope the bench measures).
- NB=53 dest blocks (not ceil(6250/128)=49): the slack lets the LPT pack cap
  every (block, half) at <=1024 edges -> CHH=8, so phase 2 runs uniform
  8-chunk octs (one DVE op per oh/prod/reduce/scale stage per oct).
- Phase-1b epilogues are split DVE (V copy) / ACT (K relu) to balance
  engines; x loads are 1024-node macro tiles to cut HWDGE issue count.
PSUM budget: mm(2) + ohtps(2) + qeps8(2) + ops(2) = 8 banks exactly.
"""
import math
import os

import numpy as np

import ml_dtypes

import concourse.bass as bass
import concourse.mybir as mybir
import concourse.tile as tile
from concourse import bacc

P = 128
C = 8                    # cores
N, F, E = 50000, 128, 800000
H, A, U = 8, 64, 128     # heads, att units, units
HD = A // H              # per-head q/k dim (8)
UD = U // H              # per-head v dim (16)
NPC = N // C             # nodes per core
NB = 53                  # dest blocks per core (>ceil(NPC/P)=49: slacker LPT
                         # pack lowers the max per-(block,half) load to <=1024
                         # -> CHH=8, so phase-2 octs are uniform 8-chunk)
HALF = N // 2            # table split point (fits int16 indices)
BF16 = mybir.dt.bfloat16
F32 = mybir.dt.float32
NP_BF16 = ml_dtypes.bfloat16
F16 = mybir.dt.float16
NP_F16 = np.float16

KVROW = 256              # bf16 elems per packed row: [K f16 (64) | V bf16 (128) | pad] = 512B


# ---------------------------------------------------------------- preprocessing
def preprocess(x, edge_index, query_kernel, query_bias, key_kernel, key_bias,
               kernel, bias):
    """Build per-core input maps + the uniform structure params.

    Destinations are assigned to (core, block) with an LPT greedy pack so
    per-block edge counts are balanced -> minimal chunk padding. Returns
    (in_maps, CHH, perm) where perm[c, b*P+i] is the global node id stored
    at output row (c, b*P+i), or -1 for unused slots.
    """
    x = np.asarray(x, np.float32)
    ei = np.asarray(edge_index, np.int64)
    row = np.concatenate([ei[0], np.arange(N, dtype=np.int64)])   # dest
    col = np.concatenate([ei[1], np.arange(N, dtype=np.int64)])   # src
    Et = row.shape[0]

    # per-core source relabeling: core c stores node n's KV row at
    # (n - off_c) mod N with off_c = c*NPC - (HALF - NPC//2), so each core's
    # self-loop sources straddle the KV0/KV1 split -> halves stay balanced.
    offs = np.array([c * NPC - (HALF - NPC // 2) for c in range(C)])
    core_e = row // NPC
    srow = (col - offs[core_e]) % N
    half_e = (srow >= HALF).astype(np.int64)
    deg0 = np.bincount(row[half_e == 0], minlength=N)
    deg1 = np.bincount(row[half_e == 1], minlength=N)
    deg = deg0 + deg1
    # --- balanced block assignment per core: greedy pack minimizing the max
    # per-(block, half) load (that max sets CHH = the gather chunk count) ---
    blk_of = np.empty(N, np.int32)
    loc_of = np.empty(N, np.int32)
    perm = np.full((C, NB * P), -1, np.int64)
    for c in range(C):
        nodes = np.arange(c * NPC, (c + 1) * NPC)
        nodes = nodes[np.argsort(-deg[nodes], kind="stable")]
        l0 = np.zeros(NB, np.int64)
        l1 = np.zeros(NB, np.int64)
        cnt = np.zeros(NB, np.int64)
        for n in nodes:
            cost = np.maximum(l0 + deg0[n], l1 + deg1[n])
            cost[cnt >= P] = 1 << 60
            b = int(np.argmin(cost))
            blk_of[n] = b
            loc_of[n] = cnt[b]
            perm[c, b * P + cnt[b]] = n
            l0[b] += deg0[n]
            l1[b] += deg1[n]
            cnt[b] += 1

    core = core_e
    lb = blk_of[row].astype(np.int64)
    ld = loc_of[row].astype(np.int64)
    half = half_e

    grp = (core * NB + lb) * 2 + half                  # [Et] in [0, C*NB*2)
    # src-sorted inside each (block, half) group: gather indices become
    # monotonic per call -> much better HBM page locality (dst order is
    # irrelevant to the device pipeline; the one-hot handles any slot order).
    order = np.argsort(grp * np.int64(N) + srow, kind="stable")
    gs = grp[order]
    counts = np.bincount(grp, minlength=C * NB * 2)
    CHH = max(1, int(math.ceil(counts.max() / P)))     # chunks per half-gather
    SPH = CHH * P                                      # slots per half
    starts = np.zeros(C * NB * 2, np.int64)
    starts[1:] = np.cumsum(counts)[:-1]
    pos = np.arange(Et) - starts[gs]
    slot = gs * SPH + pos

    idx_all = np.full(C * NB * 2 * SPH, HALF, np.int16)   # pad -> zeros row
    idx_all[slot] = (srow - half * HALF)[order].astype(np.int16)
    dest_all = np.full(C * NB * 2 * SPH, -1.0, np.float32)
    dest_all[slot] = ld[order].astype(np.float32)

    # half-major index layout so one dma_gather can cover G consecutive
    # blocks of the same half-table: [C, 16, hf, block, slot//16]
    idx_all = idx_all.reshape(C, NB, 2, CHH * 8, 16)
    idxg = np.tile(idx_all.transpose(0, 4, 2, 1, 3).reshape(C, 16, NB * 2 * CHH * 8),
                   (1, 8, 1))                              # [C, 128, 2*NB*CHH*8]
    destc = dest_all.reshape(C, NB * 2, CHH, P).transpose(0, 3, 1, 2) \
                    .reshape(C, P, NB * 2 * CHH)           # [C, 128, NB*2*CHH]

    xT = np.ascontiguousarray(x.T.astype(NP_F16))          # [128, N] fp16
    xq = np.zeros((C, P, NB * P), NP_F16)
    for c in range(C):
        valid = perm[c] >= 0
        xq[c][:, valid] = xT[:, perm[c][valid]]

    wcat = np.concatenate(
        [np.asarray(query_kernel), np.asarray(key_kernel), np.asarray(kernel)],
        axis=1).astype(NP_F16)                             # [128, 256] fp16
    qkb = np.tile(np.concatenate([np.asarray(query_bias), np.asarray(key_bias)])
                  .astype(np.float32)[None, :], (P, 1))    # [128, 128]
    outb = np.tile(np.asarray(bias, np.float32)[None, :], (P, 1))
    nobias = bool(np.all(qkb == 0.0) and np.all(outb == 0.0))

    in_maps = []
    for c in range(C):
        in_maps.append({
            "xT": np.roll(xT, -int(offs[c]), axis=1),   # table row r = node (r+off_c)%N
            "xq": xq[c], "wcat": wcat, "qkb": qkb, "outb": outb,
            "idxg": np.ascontiguousarray(idxg[c]),
            "destc": np.ascontiguousarray(destc[c]),
        })
    return in_maps, CHH, perm, nobias


# ---------------------------------------------------------------- bass program
def build_program(CHH, reps=1, nobias=False, parts="12"):
    nc = bacc.Bacc(None, target_bir_lowering=False, debug=False)

    xT = nc.dram_tensor("xT", [P, N], F16, kind="ExternalInput")
    xq = nc.dram_tensor("xq", [P, NB * P], F16, kind="ExternalInput")
    wcat_d = nc.dram_tensor("wcat", [P, 256], F16, kind="ExternalInput")
    qkb_d = nc.dram_tensor("qkb", [P, P], F32, kind="ExternalInput")
    outb_d = nc.dram_tensor("outb", [P, P], F32, kind="ExternalInput")
    idxg_d = nc.dram_tensor("idxg", [P, NB * 2 * CHH * 8], mybir.dt.int16,
                            kind="ExternalInput")
    destc_d = nc.dram_tensor("destc", [P, NB * 2 * CHH], F32, kind="ExternalInput")
    out_d = nc.dram_tensor("out", [NB * P, U], F16, kind="ExternalOutput")

    # KV tables double-buffered by rep parity so phase 1 of rep r+1 can
    # overlap phase 2 of rep r (no WAR between table generations).
    npar = min(2, reps)
    KV0s = [nc.dram_tensor(f"KV0_{p}", [HALF + 1, KVROW], BF16)
            for p in range(npar)]
    KV1s = [nc.dram_tensor(f"KV1_{p}", [N - HALF + 1, KVROW], BF16)
            for p in range(npar)]

    NCH = 2 * CHH

    with tile.TileContext(nc) as tc:
        with (
            tc.tile_pool(name="const", bufs=1) as cpool,
            tc.tile_pool(name="qx", bufs=3) as qxp,
            tc.tile_pool(name="xload", bufs=3) as xlp,
            tc.tile_pool(name="kvsb", bufs=3) as kvsbp,
            tc.tile_pool(name="kvt", bufs=2) as kvtp,
            tc.tile_pool(name="oh", bufs=4) as ohp,
            tc.tile_pool(name="oht", bufs=4) as ohtp,
            tc.tile_pool(name="wt", bufs=4) as wtp,
            tc.tile_pool(name="sc", bufs=4) as scp,
            tc.tile_pool(name="fin", bufs=4) as finp,
            tc.tile_pool(name="mm", bufs=2, space="PSUM") as mmp,
            tc.tile_pool(name="ohtps", bufs=1, space="PSUM") as ohtpsp,
            tc.tile_pool(name="qeps", bufs=3, space="PSUM") as qepsp,
            tc.tile_pool(name="ops", bufs=2, space="PSUM") as opsp,
        ):
            # resident tensors
            wcat = cpool.tile([P, 256], F16, tag="wcat")
            qkb = cpool.tile([P, P], F32, tag="qkb")
            outb = cpool.tile([P, P], F32, tag="outb")
            idxg = cpool.tile([P, NB * 2 * CHH * 8], mybir.dt.int16, tag="idxg")
            destc = cpool.tile([P, NB * 2 * CHH], F32, tag="destc")
            qalls = [cpool.tile([P, NB, A], F16, tag=f"qall{p}",
                                name=f"qall{p}") for p in range(npar)]
            iota_o = cpool.tile([P, P, 8], BF16, tag="iotao")
            iota_oi = cpool.tile([P, P, 8], mybir.dt.int32, tag="iotaoi")
            destc_b = cpool.tile([P, NB * 2 * CHH], BF16, tag="destcb")
            ident = cpool.tile([P, P], BF16, tag="ident")
            zrow = cpool.tile([1, KVROW], BF16, tag="zrow")

            nc.sync.dma_start(wcat[:], wcat_d[:])
            nc.sync.dma_start(qkb[:], qkb_d[:])
            nc.sync.dma_start(outb[:], outb_d[:])
            nc.sync.dma_start(idxg[:], idxg_d[:])
            nc.sync.dma_start(destc[:], destc_d[:])
            nc.gpsimd.iota(iota_oi[:], pattern=[[1, P], [0, 8]], base=0,
                           channel_multiplier=0)
            nc.vector.tensor_copy(iota_o[:], iota_oi[:])
            nc.vector.tensor_copy(destc_b[:], destc[:])
            from concourse.masks import make_identity
            make_identity(nc, ident[:])
            nc.vector.memset(zrow[:], 0.0)
            for p in range(npar):
                nc.sync.dma_start(KV0s[p][HALF], zrow[:])
                nc.sync.dma_start(KV1s[p][N - HALF], zrow[:])

            pools = dict(qxp=qxp, xlp=xlp, kvsbp=kvsbp, kvtp=kvtp, ohp=ohp,
                         ohtp=ohtp, wtp=wtp, scp=scp, finp=finp, mmp=mmp,
                         ohtpsp=ohtpsp, qepsp=qepsp, opsp=opsp)
            consts = dict(wcat=wcat, qkb=qkb, outb=outb, idxg=idxg,
                          destc_b=destc_b, iota_o=iota_o, ident=ident)

            for rep in range(reps):
                par = rep % npar
                _emit_rep(nc, tc, CHH, xT, xq, out_d, KV0s[par], KV1s[par],
                          qalls[par], pools, consts, rep, nobias, parts=parts)

    nc.compile()
    return nc


def _emit_rep(nc, tc, CHH, xT, xq, out_d, KV0, KV1, qall, pools, consts,
              rep, nobias=False, parts="12"):
    wcat = consts["wcat"]
    qkb = consts["qkb"]
    outb = consts["outb"]
    idxg = consts["idxg"]
    destc = consts["destc_b"]
    iota_o = consts["iota_o"]
    qxp = pools["qxp"]
    xlp = pools["xlp"]
    kvsbp = pools["kvsbp"]
    kvtp = pools["kvtp"]
    ohp = pools["ohp"]
    ohtp = pools["ohtp"]
    wtp = pools["wtp"]
    scp = pools["scp"]
    finp = pools["finp"]
    mmp = pools["mmp"]
    qepsp = pools["qepsp"]
    opsp = pools["opsp"]
    r = f"r{rep}"

    KVW = A + U              # 192 slots = 384B actually written per row

    def store_rows(r0, nr, sb_ap):
        """Store [nr, KVW] bf16 row-prefixes starting at global row r0."""
        if r0 >= HALF:
            nc.sync.dma_start(KV1[r0 - HALF: r0 - HALF + nr, 0:KVW], sb_ap)
        elif r0 + nr <= HALF:
            nc.sync.dma_start(KV0[r0: r0 + nr, 0:KVW], sb_ap)
        else:
            k = HALF - r0
            nc.sync.dma_start(KV0[r0:HALF, 0:KVW], sb_ap[0:k])
            nc.sync.dma_start(KV1[0: nr - k, 0:KVW], sb_ap[k:nr])

    # ---------------- phase 1a: Q for local nodes -> qall (f16)
    if "1" not in parts:
        _emit_p2(nc, tc, CHH, out_d, KV0, KV1, qall, pools, consts, r, nobias,
                 gathers="c" not in parts)
        return
    with nc.named_scope(f"p1a_{r}"):
        QW = 512                     # nodes per xq load
        for t in range(math.ceil(NB * P / QW)):
            n0 = t * QW
            nn = min(QW, NB * P - n0)
            ns = math.ceil(nn / P)
            xqt = qxp.tile([P, QW], F16, tag="xqt")
            nc.sync.dma_start(xqt[:, 0:nn], xq[:, n0:n0 + nn])
            for s in range(ns):
                b = t * (QW // P) + s
                qps = mmp.tile([P, 2, 192], F32, tag="mm")
                nc.tensor.matmul(qps[:, 0, 0:A], xqt[:, s * P:(s + 1) * P],
                                 wcat[:, 0:A], start=True, stop=True)
                if nobias:
                    nc.scalar.activation(qall[:, b, :], qps[:, 0, 0:A],
                                         mybir.ActivationFunctionType.Relu)
                else:
                    qtmp = finp.tile([P, A], F32, tag="qtmp")
                    nc.vector.tensor_tensor(out=qtmp[:], in0=qps[:, 0, 0:A],
                                            in1=qkb[:, 0:A],
                                            op=mybir.AluOpType.add)
                    nc.scalar.activation(qall[:, b, :], qtmp[:],
                                         mybir.ActivationFunctionType.Relu)

    # ---------------- phase 1b: full KV table (K f32-bitcast | V bf16)
    # per 1024-node macro tile: 1 x-load, 8 matmuls into 4 paired psum tiles,
    # epilogues split DVE (V copy) / ACT (K relu), 2 batched stores.
    with nc.named_scope(f"p1b_{r}"):
        XW = 1024
        NT = math.ceil(N / XW)
        for t in range(NT):
            n0 = t * XW
            nn = min(XW, N - n0)
            ns = math.ceil(nn / P)          # subtiles (8, last tile 7)
            xt = xlp.tile([P, XW], F16, tag="xt")
            nc.sync.dma_start(xt[:, 0:nn], xT[:, n0:n0 + nn])
            for hb in range(2):             # two 512-node halves per macro tile
                s0 = hb * 4
                hs = min(4, ns - s0)        # subtiles in this half
                if hs <= 0:
                    break
                hn = min(512, nn - hb * 512)
                kvsb = kvsbp.tile([P, 4, KVROW], BF16, tag="kvsb")
                pss = []
                for pair in range(math.ceil(hs / 2)):
                    ps = mmp.tile([P, 2, 192], F32, tag="mm")
                    pss.append(ps)
                    for j in range(min(2, hs - 2 * pair)):
                        s = s0 + 2 * pair + j
                        nr = min(P, nn - s * P)
                        nc.tensor.matmul(ps[0:nr, j, :],
                                         xt[:, s * P: s * P + nr],
                                         wcat[:, A:256], start=True, stop=True)
                for pair in range(math.ceil(hs / 2)):
                    np_ = min(2, hs - 2 * pair)
                    ps = pss[pair]
                    sl = slice(2 * pair, 2 * pair + np_)
                    # row layout: [K f16 (slots 0:A) | V bf16 (A:A+U) | pad]
                    if nobias:
                        nc.scalar.activation(
                            kvsb[:, sl, 0:A].bitcast(F16), ps[:, 0:np_, 0:A],
                            mybir.ActivationFunctionType.Relu)
                    else:
                        ktmp = finp.tile([P, 2, A], F32, tag="ktmp")
                        nc.vector.tensor_tensor(
                            out=ktmp[:, 0:np_, :], in0=ps[:, 0:np_, 0:A],
                            in1=qkb[:, None, A:P].broadcast_to([P, np_, A]),
                            op=mybir.AluOpType.add)
                        nc.scalar.activation(kvsb[:, sl, 0:A].bitcast(F16),
                                             ktmp[:, 0:np_, :],
                                             mybir.ActivationFunctionType.Relu)
                    # V copies all on ACT: DVE is the phase-2 bottleneck.
                    # NOTE: V as fp8e4m3 (256B rows) was tried: rel err 0.045
                    # FAILS the 2e-2 gate (concentrated softmax weights pass a
                    # single V's full quant error through) and it was not even
                    # faster (893us) — gather cost is descriptor-bound, not
                    # byte-bound. Do not retry.
                    nc.scalar.copy(kvsb[:, sl, A:A + U],
                                   ps[:, 0:np_, A:192])
                # batched store of [P, hs, KVROW]: rows n0+hb*512 + s*128 + p
                lo = n0 + hb * 512
                hi = lo + hn
                full = hn == hs * P
                if hi <= HALF or lo >= HALF:
                    dst, off = (KV0, 0) if hi <= HALF else (KV1, HALF)
                    if full:
                        nc.sync.dma_start(
                            dst[lo - off: hi - off, 0:KVW]
                            .rearrange("(s p) e -> p s e", p=P),
                            kvsb[:, 0:hs, 0:KVW])
                    else:
                        nfull = hn // P
                        if nfull:
                            nc.sync.dma_start(
                                dst[lo - off: lo - off + nfull * P, 0:KVW]
                                .rearrange("(s p) e -> p s e", p=P),
                                kvsb[:, 0:nfull, 0:KVW])
                        rem = hn - nfull * P
                        nc.sync.dma_start(
                            dst[lo - off + nfull * P: lo - off + hn, 0:KVW],
                            kvsb[0:rem, nfull, 0:KVW])
                else:
                    for s in range(hs):
                        r0 = lo + s * P
                        nr = min(P, N - r0)
                        store_rows(r0, nr, kvsb[0:nr, s, 0:KVW])

    if "2" in parts:
        _emit_p2(nc, tc, CHH, out_d, KV0, KV1, qall, pools, consts, r, nobias)


def _emit_p2(nc, tc, CHH, out_d, KV0, KV1, qall, pools, consts, r, nobias,
             gathers=True):
    idxg = consts["idxg"]
    destc = consts["destc_b"]
    iota_o = consts["iota_o"]
    ident = consts["ident"]
    outb = consts["outb"]
    kvtp = pools["kvtp"]
    ohp = pools["ohp"]
    ohtp = pools["ohtp"]
    wtp = pools["wtp"]
    scp = pools["scp"]
    finp = pools["finp"]
    ohtpsp = pools["ohtpsp"]
    qepsp = pools["qepsp"]
    opsp = pools["opsp"]
    # ---------------- phase 2: attention per dest block, oct-batched chunks
    # Gathers cover GB consecutive blocks per half-table call (amortizes the
    # ~1us SWDGE fixed cost and streams ~GB*CHH*64KB per doorbell).
    NCH = 2 * CHH
    assert CHH <= 8
    GB = 4
    with nc.named_scope(f"p2_{r}"):
        for bg in range(0, NB, GB):
            gn = min(GB, NB - bg)
            kvt = kvtp.tile([P, 2, GB * CHH, KVROW], BF16, tag="kvt")
            if gathers:
                for hf in range(2):
                    srct = KV0 if hf == 0 else KV1
                    i0 = (hf * NB + bg) * CHH * 8
                    nc.gpsimd.dma_gather(
                        kvt[:, hf, 0:gn * CHH, :], srct[:],
                        idxg[:, i0: i0 + gn * CHH * 8],
                        num_idxs=gn * CHH * P, num_idxs_reg=gn * CHH * P,
                        elem_size=KVROW, single_packet=False,
                    )
            for g in range(gn):
                b = bg + g
                ops = opsp.tile([P, U + H], F32, tag="ops")
                for o in range(2):          # oct o == half o (CHH <= 8)
                    on = CHH
                    g0 = b * NCH + o * CHH
                    kvo = kvt[:, o, g * CHH: g * CHH + on, :]
                    oh8 = ohp.tile([P, P, 8], BF16, tag="oh8")
                    nc.vector.tensor_tensor(
                        out=oh8[:, :, 0:on],
                        in0=destc[:, g0:g0 + on][:, None, :]
                            .broadcast_to([P, P, on]),
                        in1=iota_o[:, :, 0:on],
                        op=mybir.AluOpType.is_equal)
                    oht8 = ohtp.tile([P, 8, P], F16, tag="oht8")
                    qeps8 = qepsp.tile([P, 8, A], F32, tag="qeps8")
                    ohtps = ohtpsp.tile([P, 8, P], BF16, tag="ohtps")
                    for j in range(on):
                        nc.tensor.transpose(ohtps[:, j, :], oh8[:, :, j],
                                            ident[:])
                    nc.scalar.copy(oht8[:, 0:on, :], ohtps[:, 0:on, :])
                    for j in range(on):
                        nc.tensor.matmul(qeps8[:, j, :], oht8[:, j, :],
                                         qall[:, b, :], start=True, stop=True)
                    prod8 = wtp.tile([P, 8, A], F16, tag="prod8")
                    nc.vector.tensor_tensor(
                        out=prod8[:, 0:on, :], in0=qeps8[:, 0:on, :],
                        in1=kvo[:, :, 0:A].bitcast(F16),
                        op=mybir.AluOpType.mult)
                    score8 = scp.tile([P, 8, H], F16, tag="score8")
                    with nc.allow_low_precision(reason="8-elem head sum; DVE "
                                                "accumulates fp32 internally"):
                        nc.vector.tensor_reduce(
                            score8[:, 0:on, :],
                            prod8[:, 0:on, :]
                                .rearrange("p q (h d) -> p q h d", h=H),
                            axis=mybir.AxisListType.X, op=mybir.AluOpType.add)
                    wt8 = wtp.tile([P, 8, U + H], BF16, tag="wt8")
                    nc.scalar.activation(wt8[:, 0:on, U:U + H],
                                         score8[:, 0:on, :],
                                         mybir.ActivationFunctionType.Exp)
                    # NOTE: expanding the exp weights 16-wide on ACT to get
                    # DVE 2x packing here was tried and REGRESSED (930us vs
                    # 877): the ACT copy costs more than the DVE 2x saves.
                    nc.vector.tensor_tensor(
                        out=wt8[:, 0:on, 0:U].rearrange("p q (h u) -> p q h u",
                                                        h=H),
                        in0=kvo[:, :, A:A + U]
                            .rearrange("p q (h u) -> p q h u", h=H),
                        in1=wt8[:, 0:on, U:U + H][:, :, :, None]
                            .broadcast_to([P, on, H, UD]),
                        op=mybir.AluOpType.mult)
                    for j in range(on):
                        ch = o * CHH + j
                        nc.tensor.matmul(ops[:], oh8[:, :, j], wt8[:, j, :],
                                         start=(ch == 0), stop=(ch == NCH - 1))
                recip = finp.tile([P, H], F32, tag="recip")
                nc.vector.reciprocal(recip[:], ops[:, U:U + H])
                o1 = finp.tile([P, U], F16, tag="o1")
                nc.vector.tensor_tensor(
                    out=o1[:].rearrange("p (h u) -> p h u", h=H),
                    in0=ops[:, 0:U].rearrange("p (h u) -> p h u", h=H),
                    in1=recip[:][:, :, None].broadcast_to([P, H, UD]),
                    op=mybir.AluOpType.mult)
                if not nobias:
                    nc.vector.tensor_tensor(out=o1[:], in0=o1[:], in1=outb[:],
                                            op=mybir.AluOpType.add)
                nc.sync.dma_start(out_d[b * P: (b + 1) * P], o1[:])


# ---------------------------------------------------------------- execution
class SpmdRunner:
    def __init__(self, nc, n_cores=C):
        import jax
        from jax.sharding import Mesh, PartitionSpec
        from jax.experimental.shard_map import shard_map
        from concourse.bass2jax import (_bass_exec_p, install_neuronx_cc_hook,
                                        partition_id_tensor)
        install_neuronx_cc_hook()
        self.jax = jax
        self.nc = nc
        self.n_cores = n_cores
        partition_name = nc.partition_id_tensor.name if nc.partition_id_tensor else None
        in_names, out_names, out_avals = [], [], []
        for alloc in nc.m.functions[0].allocations:
            if not isinstance(alloc, mybir.MemoryLocationSet):
                continue
            name = alloc.memorylocations[0].name
            if alloc.kind == "ExternalInput":
                if name != partition_name:
                    in_names.append(name)
            elif alloc.kind == "ExternalOutput":
                out_names.append(name)
                out_avals.append(jax.core.ShapedArray(
                    tuple(alloc.tensor_shape), mybir.dt.np(alloc.dtype)))
        self.in_names, self.out_names, self.out_avals = in_names, out_names, out_avals
        n_params = len(in_names)

        all_in_names = list(in_names) + list(out_names)
        if partition_name is not None:
            all_in_names.append(partition_name)

        def _body(*args):
            operands = list(args)
            if partition_name is not None:
                operands.append(partition_id_tensor())
            outs = _bass_exec_p.bind(
                *operands,
                out_avals=tuple(out_avals),
                in_names=tuple(all_in_names),
                out_names=tuple(out_names),
                lowering_input_output_aliases=(),
                sim_require_finite=False,
                sim_require_nnan=False,
                nc=nc,
            )
            return tuple(outs)

        devices = jax.devices()[:n_cores]
        self.mesh = Mesh(np.asarray(devices), ("core",))
        n_extra = len(out_names)
        in_specs = (PartitionSpec("core"),) * (n_params + n_extra)
        out_specs = (PartitionSpec("core"),) * len(out_names)
        self.fn = jax.jit(
            shard_map(_body, mesh=self.mesh, in_specs=in_specs,
                      out_specs=out_specs, check_rep=False),
            keep_unused=True,
        )

    def put_inputs(self, in_maps):
        from jax.sharding import NamedSharding, PartitionSpec
        sharding = NamedSharding(self.mesh, PartitionSpec("core"))
        args = []
        for name in self.in_names:
            concat = np.concatenate([np.asarray(m[name]) for m in in_maps], axis=0)
            args.append(self.jax.device_put(concat, sharding))
        for av in self.out_avals:
            args.append(self.jax.device_put(
                np.zeros((self.n_cores * av.shape[0], *av.shape[1:]), av.dtype),
                sharding))
        return args

    def __call__(self, args):
        outs = self.fn(*args)
        self.jax.block_until_ready(outs)
        return outs

    def run_to_numpy(self, args):
        outs = self(args)
        res = []
        for c in range(self.n_cores):
            d = {}
            for i, name in enumerate(self.out_names):
                d[name] = np.asarray(outs[i]).reshape(
                    self.n_cores, *self.out_avals[i].shape)[c]
            res.append(d)
        return res


_CACHE = {}


def _get_runner(CHH, reps=1, nobias=False, parts="12"):
    key = (CHH, reps, nobias, parts)
    if key not in _CACHE:
        nc = build_program(CHH, reps=reps, nobias=nobias, parts=parts)
        _CACHE[key] = SpmdRunner(nc)
    return _CACHE[key]


def kernel(x, edge_index, query_kernel, query_bias, key_kernel, key_bias,
           kernel, bias):
    in_maps, CHH, perm, nobias = preprocess(x, edge_index, query_kernel,
                                            query_bias, key_kernel, key_bias,
                                            kernel, bias)
    runner = _get_runner(CHH, nobias=nobias)
    args = runner.put_inputs(in_maps)
    res = runner.run_to_numpy(args)
    out = np.empty((N, U), np.float32)
    for c in range(C):
        valid = perm[c] >= 0
        out[perm[c][valid]] = res[c]["out"][valid]
    return out

